# revision 32
# baseline (speedup 1.0000x reference)
"""Trainium2 Bass kernel: banded additive attention (window 64).

reference semantics (B=4, T=1024, D=512, U=32, WIDTH=64):
  q = x @ Wt ; k = x @ Wx
  e[b,t,j] = exp(Wa . tanh(q[b,t]+k[b,j]+bh) + ba) for j in [t-32, t+31]
  a = e / (sum_j e + 1e-7) ; v = a @ x

Sharding: 8 NeuronCores = (batch b, T-half). Each core computes 512 query
rows with a 32-row halo; weights replicated. Inputs are packed/cast on the
host into SBUF-shaped blocks so each needs a single DMA.

Per-core pipeline (Tile-scheduled), v3: the 512 query columns run as two
independent 256-col halves; within each half the exp->shear->value tail
runs as two 128-col pieces with per-piece tiles so the Tile scheduler can
pipeline them. Emission order interleaves half-0's tail ahead of half-1's
scores so engine priorities keep ACT (the bottleneck: ~7us of tanh)
saturated while PE/DVE drain half-0 underneath, and half 1 uses a small
final tanh batch so its tail starts early.

  0. Input DMAs spread over all 4 DGE queues (desc-gen parallelism),
     value-side xe last; PE-warmup matmuls beat the p-state ramp.
  1. Projections, chunk-pipelined against the x^T chunk DMAs; kT in two
     column pieces (DVE+ACT copies), shifted-replication matmuls into
     diagonal k4 in three pieces (DVE copies), q4 via host-replicated Wt
     (ACT copies, per-half pieces); bh rides the tanh bias operand.
  2. Scores per half: DVE add (diag layout) -> ACT tanh -> accumulating
     PE matmuls with sliced wide-Wa lhsT -> E[64,256] (+rank-32 edge
     masks).
  3. Tail per half, two 128-col pieces: ACT exp (bias=ba) into sigmaA
     order -> radix-8x8 shear butterfly (PE shift-matmuls; permutes ride
     on DVE strided copies) -> banded Bsb piece -> value matmuls against
     xe blocks (ones column yields the softmax denominator) -> DVE
     reciprocal -> DVE/ACT scale -> per-piece 128-row output DMA.
"""
import os
import sys

sys.path.insert(0, "/opt/trn_rl_repo")

import numpy as np
import ml_dtypes  # noqa: E402
import concourse.bass as bass  # noqa: E402
import concourse.mybir as mybir  # noqa: E402
from concourse import bacc, tile  # noqa: E402
from concourse.ap import AP  # noqa: E402
from concourse.bass_utils import run_bass_kernel_spmd  # noqa: E402

F32 = mybir.dt.float32
BF16 = mybir.dt.bfloat16
ActFn = mybir.ActivationFunctionType

B, T, D, U = 4, 1024, 512, 32
WIDTH = 64
EPS = 1e-7
T_LOC = 512
TH = 256   # half width
TP = 128   # piece width
HALO = 576
NBLK = 8
NCORES = 8

_CDT = BF16 if os.environ.get("ATTN_CDT", "bf16") == "bf16" else F32
_NWARM = int(os.environ.get("ATTN_WARM", "4"))

# kT column split: piece1 [0:KS1], piece2 [KS1-4:576] (4-col overlap so
# shift matmuls for k4 cols [KS1-4:512] read only piece-2's tile)
KS1 = 292


def _np_dt(cdt):
    return ml_dtypes.bfloat16 if cdt == BF16 else np.float32


def _emit(nc, tc, cdt, xt, xe, wws, mbb, vout):
    from contextlib import ExitStack
    ctx = ExitStack()
    with ctx:
        cpool = ctx.enter_context(tc.tile_pool(name="consts", bufs=1))
        work = ctx.enter_context(tc.tile_pool(name="work", bufs=1))
        tpool = ctx.enter_context(tc.tile_pool(name="tanh", bufs=2))
        bpool = ctx.enter_context(tc.tile_pool(name="bfly", bufs=4))
        opool = ctx.enter_context(tc.tile_pool(name="outs", bufs=2))
        rpool = ctx.enter_context(tc.tile_pool(name="rcols", bufs=4))

        # ---------- Phase 0: ACT table preload, DMAs, PE warmup ----------
        dummy = cpool.tile([1, 1], F32, tag="dummy")
        nc.vector.memset(dummy[:], 0.0)
        nc.scalar.activation(dummy[:], dummy[:], ActFn.Exp)

        wws_sb = cpool.tile([128, 1147], cdt, tag="wws_sb")
        xtc = [cpool.tile([128, HALO], cdt, tag=f"xt{c}", name=f"xt{c}")
               for c in range(4)]
        mbb_sb = cpool.tile([128, 2], F32, tag="mbb_sb")
        xe_all = cpool.tile([128, 8 * 513], cdt, tag="xe_all")
        # Spread desc-gen over all four DGE queues; within each queue the
        # second DMA's descriptors generate behind the first, keeping the
        # shared HBM track in need order (xe, needed last, lands last).
        nc.sync.dma_start(wws_sb[:, 0:640], wws[:, 0:640])
        nc.scalar.dma_start(xtc[0][:], xt[:, 0:HALO])
        nc.gpsimd.dma_start(xtc[1][:], xt[:, HALO:2 * HALO])
        nc.sync.dma_start(xtc[2][:], xt[:, 2 * HALO:3 * HALO])
        nc.gpsimd.dma_start(xtc[3][:], xt[:, 3 * HALO:])
        nc.sync.dma_start(wws_sb[:, 640:1147], wws[:, 640:1147])
        nc.scalar.dma_start(mbb_sb[:], mbb[:])
        nc.scalar.dma_start(xe_all[:], xe[:])
        CORDER = (1, 0, 2, 3)  # chunk arrival order given the queues above

        w_all = wws_sb[:, 0:640]
        wa_sb = wws_sb[:, 640:764]
        # sh: [128, 255] 0/1 band, sh[k, c] = (c == k + 127)
        sh_sb = wws_sb[:, 764:1019]
        # edge-mask rank-32 factors: R_lo/R_hi [32, 32] in cdt
        rlo_sb = wws_sb[0:32, 1019:1051]
        rhi_sb = wws_sb[0:32, 1051:1083]
        ba_sb = mbb_sb[0:64, 0:1]
        bh4_sb = mbb_sb[:, 1:2]

        if _NWARM:
            warm = cpool.tile([128, 512], cdt, tag="warm")
            nc.vector.memset(warm[:], 0.0)
            with tc.tile_pool(name="wps", bufs=1, space="PSUM") as wps:
                wp = wps.tile([128, 512], F32, tag="wp")
                for i in range(_NWARM):
                    nc.tensor.matmul(wp[:], warm[:, 0:128], warm[:],
                                     start=True, stop=True)

        # ---------- Phase 1: projections (chunk- and piece-pipelined) ----
        # pp1 closes (explicitly, LIFO after escore opens first) after the
        # last k4-piece copy emission so its banks free up for the
        # stage/value pools; the copies still interleave with half-0's
        # score batches in priority order.
        spool = ctx.enter_context(
            tc.tile_pool(name="escore", bufs=2, space="PSUM"))
        pp1_cm = tc.tile_pool(name="pp1", bufs=1, space="PSUM")
        pp1 = pp1_cm.__enter__()
        kTa_ps = pp1.tile([U, KS1], F32, tag="kTa")
        kTb_ps = pp1.tile([U, HALO - KS1], F32, tag="kTb")
        q4_ps = pp1.tile([128, T_LOC], F32, tag="q4_ps")
        for i, c in enumerate(CORDER):
            wx = w_all[:, 512 + 32 * c:512 + 32 * c + 32]
            st, sp = (i == 0), (i == 3)
            nc.tensor.matmul(kTa_ps[:], wx, xtc[c][:, 0:KS1],
                             start=st, stop=sp)
            nc.tensor.matmul(kTb_ps[:], wx, xtc[c][:, KS1:HALO],
                             start=st, stop=sp)
        # q4 after all kT matmuls: the k-chain (kT -> shifts -> k4) gates
        # the first tanh, so the last-arriving chunk feeds kT first.
        for i, c in enumerate(CORDER):
            nc.tensor.matmul(q4_ps[:], w_all[:, 128 * c:128 * c + 128],
                             xtc[c][:, 32:32 + T_LOC],
                             start=(i == 0), stop=(i == 3))
        # kT staged to SBUF in two pieces (DVE + ACT); piece 2 holds
        # cols [KS1-4 : 576] so shift reads never span tiles.
        kT1 = work.tile([U, KS1], cdt, tag="kT1")
        nc.vector.tensor_copy(kT1[:], kTa_ps[:])
        kT2 = work.tile([U, HALO - KS1 + 4], cdt, tag="kT2")
        nc.vector.tensor_copy(kT2[:, 0:4], kTa_ps[:, KS1 - 4:KS1])
        nc.scalar.copy(kT2[:, 4:], kTb_ps[:])
        # q4 to SBUF per half (ACT); bh is added by the tanh bias.
        q4s = work.tile([128, T_LOC], cdt, tag="q4s")
        nc.scalar.copy(q4s[:, 0:TH], q4_ps[:, 0:TH])
        nc.scalar.copy(q4s[:, TH:T_LOC], q4_ps[:, TH:T_LOC])
        # k4[32g+u, c] = kT[u, c+g]: shifted-replication matmuls
        # (lhsT = sh slices), three column pieces in separate PSUM
        # tiles so each piece's copy waits only its own shifts.
        KB = KS1 - 4  # 288
        k4p0 = pp1.tile([128, KB], F32, tag="k4p0")
        # pieces 1+2 share one bank-fitting tile ((224+61)*4B < 2KB);
        # neither region crosses a bank boundary.
        k4p12 = pp1.tile([128, 512 - KB + 61], F32, tag="k4p12")
        for g in range(4):
            lhsT = sh_sb[0:32, 127 - 32 * g:255 - 32 * g]
            nc.tensor.matmul(k4p0[:], lhsT, kT1[:, g:g + KB],
                             start=(g == 0), stop=(g == 3))
        for g in range(4):
            lhsT = sh_sb[0:32, 127 - 32 * g:255 - 32 * g]
            nc.tensor.matmul(k4p12[:, 0:512 - KB], lhsT,
                             kT2[:, g:g + (512 - KB)],
                             start=(g == 0), stop=(g == 3))
        for g in range(4):
            lhsT = sh_sb[0:32, 127 - 32 * g:255 - 32 * g]
            nc.tensor.matmul(k4p12[:, 512 - KB:512 - KB + 61], lhsT,
                             kT2[:, 512 - KB + g:573 - KB + g],
                             start=(g == 0), stop=(g == 3))
        k4 = work.tile([128, HALO], cdt, tag="k4")

        E_tiles = [None, None]

        def new_E(h):
            E_ps = spool.tile([64, TH], F32, tag="E", name=f"E{h}")
            E_tiles[h] = E_ps
            return E_ps

        def emit_batch(h, E_ps, s0, nsl):
            tin = tpool.tile([128, TH * nsl], cdt, tag="tin",
                             name=f"tin{h}_{s0}")
            k4ap = AP(k4[:].tensor, 4 * s0 + TH * h,
                      [[HALO, 128], [4, nsl], [1, TH]])
            q4ap = AP(q4s[:].tensor, TH * h,
                      [[T_LOC, 128], [0, nsl], [1, TH]])
            nc.vector.tensor_add(
                tin[:].rearrange("p (a t) -> p a t", a=nsl),
                q4ap, k4ap)
            tout = tpool.tile([128, TH * nsl], cdt, tag="tout",
                              name=f"tout{h}_{s0}")
            nc.scalar.activation(tout[:], tin[:], ActFn.Tanh,
                                 bias=bh4_sb)
            for j in range(nsl):
                r = s0 + j
                nc.tensor.matmul(E_ps[:],
                                 wa_sb[:, 60 - 4 * r:124 - 4 * r],
                                 tout[:, TH * j:TH * j + TH],
                                 start=(r == 0), stop=(r == 15))

        def emit_edge(h, E_ps):
            # edge mask as rank-32 accumulation: E += -30 on invalid j.
            # Emitted after the first batch (accumulation commutes) so the
            # final r=15 matmul, not this one, closes the accumulation.
            if h == 0:
                nc.tensor.matmul(E_ps[:, 0:32], sh_sb[0:32, 127:191],
                                 rlo_sb, start=False, stop=False)
            else:
                nc.tensor.matmul(E_ps[:, 224:256], sh_sb[0:32, 95:159],
                                 rhi_sb, start=False, stop=False)

        B0_tiles = {}

        def tail_exp(h, p):
            """exp for 128-col piece p of half h (sigmaA order).

            piece-local t' = 64mm + 8a + b (mm in {0,1});
            B0 col 16b+8mm+a -> shift b -> S1 col 16a+8mm+b -> shift 8a
            -> Bsb col 64mm+8a+b."""
            E_ps = E_tiles[h]
            B0 = bpool.tile([128, TP], cdt, tag="B0", name=f"B0_{h}{p}")
            B0_tiles[(h, p)] = B0
            nc.gpsimd.memset(B0[64:128, :], 0.0)
            b0_out = AP(B0[:].tensor, 0,
                        [[TP, 64], [16, 8], [8, 2], [1, 8]])
            e_in = AP(E_ps[:].tensor, TP * p,
                      [[TH, 64], [1, 8], [64, 2], [8, 8]])
            nc.scalar.activation(b0_out, e_in, ActFn.Exp, bias=ba_sb)

        def tail_bfly(h, p, s1_eng="dve", bsb_eng="dve"):
            B0 = B0_tiles[(h, p)]
            P1 = stpool.tile([128, TP], F32, tag="stg", name=f"P1_{h}{p}")
            for b in range(8):
                nc.tensor.matmul(P1[:, 16 * b:16 * b + 16],
                                 sh_sb[:, 127 - b:255 - b],
                                 B0[:, 16 * b:16 * b + 16],
                                 start=True, stop=True)
            S1 = bpool.tile([128, TP], cdt, tag="S1", name=f"S1_{h}{p}")
            s1_out = AP(S1[:].tensor, 0,
                        [[TP, 128], [16, 8], [8, 2], [1, 8]])
            p1_in = AP(P1[:].tensor, 0,
                       [[TP, 128], [1, 8], [8, 2], [16, 8]])
            if s1_eng == "act":
                nc.scalar.copy(s1_out, p1_in)
            else:
                nc.vector.tensor_copy(s1_out, p1_in)
            P2 = stpool.tile([128, TP], F32, tag="stg", name=f"P2_{h}{p}")
            for a in range(8):
                nc.tensor.matmul(P2[:, 16 * a:16 * a + 16],
                                 sh_sb[:, 127 - 8 * a:255 - 8 * a],
                                 S1[:, 16 * a:16 * a + 16],
                                 start=True, stop=True)
            Bsb = bpool.tile([128, TP], cdt, tag="Bsb", name=f"Bsb_{h}{p}")
            bsb_out = AP(Bsb[:].tensor, 0,
                         [[TP, 128], [64, 2], [8, 8], [1, 8]])
            p2_in = AP(P2[:].tensor, 0,
                       [[TP, 128], [8, 2], [16, 8], [1, 8]])
            if bsb_eng == "act":
                nc.scalar.copy(bsb_out, p2_in)
            else:
                nc.vector.tensor_copy(bsb_out, p2_in)
            return Bsb

        vo_tiles = [None, None]
        vpr = {}

        def value_den(h, p, Bsb):
            """Denominator via one N=1 matmul (lhsT=Bsb, rhs=xe's ones
            column; Bsb column order IS t', matching vp partitions), so
            the reciprocal overlaps the value matmuls; then the two
            512-wide value matmuls."""
            vp = vpool.tile([128, 512], F32, tag="vp", name=f"vp{h}{p}")
            vden = vpool.tile([128, 1], F32, tag="vden", name=f"vd{h}{p}")
            nc.tensor.matmul(vden[:], Bsb[:], xe_all[:, 512:513],
                             start=True, stop=True)
            # EPS dropped: s >= 64*exp(-~5) makes 1e-7 negligible
            rcol = rpool.tile([128, 1], F32, tag="rcol", name=f"rc{h}{p}")
            nc.vector.reciprocal(rcol[:], vden[:])
            for mm in range(2):
                m = 4 * h + 2 * p + mm
                lhsT = Bsb[:, 64 * mm:64 * mm + 64]
                rhs = xe_all[:, 513 * m:513 * m + 513]
                nc.tensor.matmul(vp[64 * mm:64 * mm + 64, :],
                                 lhsT, rhs[:, 0:512],
                                 start=True, stop=True)
            vpr[(h, p)] = (vp, rcol)

        def scale_dma(h, p, scale_eng):
            if vo_tiles[h] is None:
                vo_tiles[h] = opool.tile([128, 1024], vout.dtype,
                                         tag="vo", name=f"vo{h}")
            vo = vo_tiles[h]
            vp, rcol = vpr[(h, p)]
            if scale_eng == "act":
                nc.scalar.activation(vo[:, 512 * p:512 * p + 512], vp[:],
                                     ActFn.Copy, scale=rcol[:])
            else:
                nc.vector.tensor_scalar_mul(vo[:, 512 * p:512 * p + 512],
                                            vp[:], rcol[:])
            if p == 1:  # one 3D DMA per half once both pieces are scaled
                dst = AP(vout[:].tensor, TH * h * D,
                         [[D, 128], [TP * D, 2], [1, D]])
                src = AP(vo[:].tensor, 0,
                         [[1024, 128], [512, 2], [1, 512]])
                nc.sync.dma_start(dst, src)

        # Emission order drives engine priority: k4-piece copies land just
        # ahead of the score batches that need them (adds outrank later
        # copies on DVE); half-0 tail-head precedes half-1 scores so
        # exp-h0 preempts between h1 tanh batches; half-1 scores precede
        # value-h0 so E-h1 matmuls outrank it on PE.
        E0 = new_E(0)
        nc.vector.tensor_copy(k4[:, 0:KB], k4p0[:])
        emit_batch(0, E0, 0, 2)
        emit_edge(0, E0)
        emit_batch(0, E0, 2, 6)
        nc.vector.tensor_copy(k4[:, KB:512], k4p12[:, 0:512 - KB])
        emit_batch(0, E0, 8, 8)
        # tail cols feed only half 1; ACT copies them off the shared tile
        nc.scalar.copy(k4[:, 512:573], k4p12[:, 512 - KB:512 - KB + 61])
        pp1_cm.__exit__(None, None, None)
        stpool = ctx.enter_context(
            tc.tile_pool(name="stage", bufs=2, space="PSUM"))
        vpool = ctx.enter_context(
            tc.tile_pool(name="vpsum", bufs=2, space="PSUM"))
        tail_exp(0, 0)
        tail_exp(0, 1)
        bsb00 = tail_bfly(0, 0)
        value_den(0, 0, bsb00)
        scale_dma(0, 0, "dve")
        bsb01 = tail_bfly(0, 1)
        value_den(0, 1, bsb01)
        E1 = new_E(1)
        emit_batch(1, E1, 0, 8)
        emit_edge(1, E1)
        emit_batch(1, E1, 8, 6)
        emit_batch(1, E1, 14, 2)
        tail_exp(1, 0)
        tail_exp(1, 1)
        bsb10 = tail_bfly(1, 0)
        bsb11 = tail_bfly(1, 1, s1_eng="act")
        value_den(1, 0, bsb10)
        scale_dma(1, 0, "dve")
        value_den(1, 1, bsb11)
        scale_dma(0, 1, "act")
        scale_dma(1, 1, "act")


def build_nc(cdt=_CDT):
    nc = bacc.Bacc("TRN2", target_bir_lowering=False)
    xt = nc.dram_tensor("xt", [128, 4 * HALO], cdt, kind="ExternalInput")
    xe = nc.dram_tensor("xe", [128, 8 * 513], cdt, kind="ExternalInput")
    wws = nc.dram_tensor("wws", [128, 1147], cdt, kind="ExternalInput")
    mbb = nc.dram_tensor("mbb", [128, 2], F32, kind="ExternalInput")
    vout = nc.dram_tensor("v", [T_LOC, D], _CDT if os.environ.get("ATTN_VOUT", "bf16") == "bf16" else F32, kind="ExternalOutput")
    with tile.TileContext(nc) as tc:
        _emit(nc, tc, cdt, xt, xe, wws, mbb, vout)
    nc.compile()
    return nc


# ---------------- host-side prep ----------------

def prep_core_inputs(x, Wt, Wx, bh, Wa, ba, core, cdt=_CDT):
    ndt = _np_dt(cdt)
    b, half = core // 2, core % 2
    t0 = half * T_LOC
    lo, hi = t0 - 32, t0 + 544
    pad_lo, pad_hi = max(0, -lo), max(0, hi - T)
    xs = x[b, max(0, lo):min(T, hi), :]
    x_halo = np.pad(xs, ((pad_lo, pad_hi), (0, 0)))     # [576, 512]

    # xt: [128, 4*576], chunk c = x_halo[:, 128c:128c+128].T
    xt = np.empty((128, 4 * HALO), np.float32)
    for c in range(4):
        xt[:, HALO * c:HALO * (c + 1)] = x_halo[:, 128 * c:128 * c + 128].T
    # xe: [128, 8*513], block m = rows [64m, 64m+128) with ones column
    xe_rows = np.concatenate(
        [x_halo, np.ones((HALO, 1), np.float32)], 1)    # [576, 513]
    xe = np.empty((128, 8 * 513), np.float32)
    for m in range(NBLK):
        xe[:, 513 * m:513 * (m + 1)] = xe_rows[64 * m:64 * m + 128, :]
    # wws: [128, 1147] = w[640] | wa_wide[124] | sh[255] | Rlo[32] | Rhi[32]
    #                    | spare[64]
    wws = np.zeros((128, 1147), np.float32)
    for c in range(4):
        wws[:, 128 * c:128 * c + 128] = np.tile(Wt[128 * c:128 * c + 128, :],
                                                (1, 4))
        wws[:, 512 + 32 * c:512 + 32 * c + 32] = Wx[128 * c:128 * c + 128, :]
    for g in range(4):
        wws[32 * g:32 * g + 32, 640 + 60 + g] = Wa[:, 0]
    kk = np.arange(128)
    wws[kk, 764 + kk + 127] = 1.0
    # edge-mask factors: E[d', t] += -30 where j = t0 + t + d' - 32 invalid.
    # left edge (t0 == 0):  invalid iff t + d' < 32  (d' = k in [0,32))
    # right edge (t0+512 == T): invalid iff t + d' > 543 (d' = k+32)
    ks = np.arange(32)[:, None]
    ts = np.arange(32)[None, :]
    if t0 == 0:
        wws[0:32, 1019:1051] = np.where(ts < 32 - ks, -30.0, 0.0)
    if t0 + T_LOC == T:
        wws[0:32, 1051:1083] = np.where((480 + ts) + (ks + 32) > 543,
                                        -30.0, 0.0)
    # mbb: [128, 2] = ba (rows 0-63) | bh4
    mbb = np.zeros((128, 2), np.float32)
    mbb[0:64, 0] = float(np.asarray(ba).reshape(-1)[0])
    mbb[:, 1] = np.tile(np.asarray(bh, np.float32), 4)

    return {
        "xt": xt.astype(ndt),
        "xe": xe.astype(ndt),
        "wws": wws.astype(ndt),
        "mbb": mbb,
    }


_NC_CACHE = {}


def _get_nc(cdt=_CDT):
    key = str(cdt)
    if key not in _NC_CACHE:
        _NC_CACHE[key] = build_nc(cdt)
    return _NC_CACHE[key]


def kernel(x, Wt, Wx, bh, Wa, ba, _trace=False):
    x = np.asarray(x, np.float32)
    Wt = np.asarray(Wt, np.float32)
    Wx = np.asarray(Wx, np.float32)
    bh = np.asarray(bh, np.float32)
    Wa = np.asarray(Wa, np.float32)
    ba = np.asarray(ba, np.float32)
    nc = _get_nc()
    in_maps = [prep_core_inputs(x, Wt, Wx, bh, Wa, ba, c)
               for c in range(NCORES)]
    res = run_bass_kernel_spmd(nc, in_maps, core_ids=list(range(NCORES)),
                               trace=_trace)
    out = np.empty((B, T, D), np.float32)
    for c in range(NCORES):
        b, half = c // 2, c % 2
        out[b, half * T_LOC:(half + 1) * T_LOC, :] = np.asarray(
            res.results[c]["v"], np.float32)
    if _trace:
        return out, res
    return out
